# revision 3
# baseline (speedup 1.0000x reference)
"""BiGRU (nn_BiGRU) Trainium2 Bass kernel.

Sharding: 8 NeuronCores = 2 directions x 4 batch-quarters (8 sequences each).
The reference's "backward" cell also scans forward over time, so both
directions run the identical program on different weights/batch slices.

Per core:
  phase 1: XPT[n, t*8+b] = (x @ Wi + bi + bh(r,z))^T via fp32r GEMMs
           (hidden-major output so the recurrence elementwise runs with
           128 active partitions).
  phase 2: 2048-step GRU recurrence. Wh is the bf16 *stationary* matmul
           operand so each step's gate pre-activations come out already
           hidden-major in PSUM ([128, kt, batch]); no per-step transpose.
           The hidden state is carried twice: bf16 (next step's matmul
           operand, critical path) and fp32 (output + z*h term).
"""
import sys

sys.path.insert(0, "/opt/trn_rl_repo")
import numpy as np

S, B, I, H = 2048, 32, 512, 512
KT = 4    # contraction tiles of 128
NT = 12   # gate-dim tiles of 128 (3 gates x 4)
BL = 8    # sequences per core
T = 128   # recurrence steps per block
NCORES = 8

_CACHE = {}


def _build_program():
    import concourse.bass as bass
    import concourse.mybir as mybir
    import concourse.tile as tile
    from concourse import bacc
    from contextlib import ExitStack

    F32 = mybir.dt.float32
    F32R = mybir.dt.float32r
    BF16 = mybir.dt.bfloat16
    AF = mybir.ActivationFunctionType

    nblocks = S // T
    M = S * BL
    MCHUNK = 512
    nmchunks = M // MCHUNK
    TB = T * BL

    nc = bacc.Bacc("TRN2", target_bir_lowering=False, debug=False,
                   num_devices=NCORES)

    xT = nc.dram_tensor("xT", [KT, 128, M], F32R, kind="ExternalInput")
    wi = nc.dram_tensor("wi", [KT, 128, 3 * H], F32R, kind="ExternalInput")
    whT = nc.dram_tensor("whT", [KT, 128, 3 * H], BF16, kind="ExternalInput")
    bias_i = nc.dram_tensor("bias_i", [128, NT], F32, kind="ExternalInput")
    bhn_bc = nc.dram_tensor("bhn_bc", [128, KT, BL], F32, kind="ExternalInput")
    houtT = nc.dram_tensor("houtT", [KT, 128, S, BL], F32, kind="ExternalOutput")

    with tile.TileContext(nc) as tc, ExitStack() as stk:
        const_pool = stk.enter_context(tc.tile_pool(name="const", bufs=1))
        wh_sb = const_pool.tile([128, KT, 3 * H], BF16)
        bias_sb = const_pool.tile([128, NT], F32)
        bhn_sb = const_pool.tile([128, KT, BL], F32)
        hT_bf = const_pool.tile([128, KT, BL], BF16)
        hz_f32 = const_pool.tile([128, KT, BL], F32)
        for kt in range(KT):
            nc.sync.dma_start(out=wh_sb[:, kt, :], in_=whT[kt])
        nc.sync.dma_start(out=bias_sb, in_=bias_i[:])
        nc.sync.dma_start(out=bhn_sb, in_=bhn_bc[:])
        nc.vector.memset(hT_bf, 0.0)
        nc.vector.memset(hz_f32, 0.0)

        dram_pool = stk.enter_context(
            tc.tile_pool(name="dramp", bufs=1, space="DRAM"))
        xpt_dram = dram_pool.tile([NT, 128, M + 2 * TB], F32)

        # ---------------- phase 1: input projections ----------------
        with tc.tile_pool(name="p1wi", bufs=1) as wi_pool, \
             tc.tile_pool(name="p1x", bufs=3) as xin_pool, \
             tc.tile_pool(name="p1o", bufs=4) as xo_pool, \
             tc.tile_pool(name="p1ps", bufs=2, space="PSUM") as ps_pool:
            wi_sb = wi_pool.tile([128, KT, 3 * H], F32R)
            for kt in range(KT):
                nc.sync.dma_start(out=wi_sb[:, kt, :], in_=wi[kt])
            with tc.For_i(0, nmchunks, 2, name="p1") as ci:
                for u in range(2):
                    moff = (ci + u) * MCHUNK
                    xc = xin_pool.tile([128, KT, MCHUNK], F32R, tag="xc")
                    for kt in range(KT):
                        nc.sync.dma_start(
                            out=xc[:, kt, :], in_=xT[kt, :, bass.ds(moff, MCHUNK)]
                        )
                    for nt in range(NT):
                        ps = ps_pool.tile([128, MCHUNK], F32, tag="ps")
                        for kt in range(KT):
                            nc.tensor.matmul(
                                ps,
                                lhsT=wi_sb[:, kt, nt * 128:(nt + 1) * 128],
                                rhs=xc[:, kt, :],
                                start=(kt == 0),
                                stop=(kt == KT - 1),
                            )
                        xo = xo_pool.tile([128, MCHUNK], F32, tag="xo")
                        nc.vector.tensor_scalar_add(xo, ps, bias_sb[:, nt:nt + 1])
                        nc.sync.dma_start(
                            out=xpt_dram[nt, :, bass.ds(moff, MCHUNK)], in_=xo
                        )

        # ---------------- phase 2: recurrence ----------------
        with tc.tile_pool(name="p2x", bufs=1) as xpt_pool, \
             tc.tile_pool(name="p2o", bufs=1) as hout_pool, \
             tc.tile_pool(name="p2e", bufs=2) as ew_pool, \
             tc.tile_pool(name="p2ps", bufs=2, space="PSUM") as gps_pool:

            xpt_a = xpt_pool.tile([128, NT, TB], F32, tag="xpa")
            xpt_b = xpt_pool.tile([128, NT, TB], F32, tag="xpb")
            hout_a = hout_pool.tile([128, KT, T, BL], F32, tag="hoa")
            hout_b = hout_pool.tile([128, KT, T, BL], F32, tag="hob")

            def load_block(dst, start_col):
                for nt in range(NT):
                    nc.sync.dma_start(
                        out=dst[:, nt, :],
                        in_=xpt_dram[nt, :, bass.ds(start_col, TB)],
                    )

            def compute_block(xpt, hout, prev_hout, bi):
                for t in range(T):
                    hprev_f = (prev_hout[:, :, T - 1, :] if t == 0
                               else hout[:, :, t - 1, :])
                    gt_n = gps_pool.tile([128, KT, BL], F32, tag="gt_n")
                    gt_r = gps_pool.tile([128, KT, BL], F32, tag="gt_r")
                    gt_z = gps_pool.tile([128, KT, BL], F32, tag="gt_z")
                    for dst, base in ((gt_n, 2 * KT), (gt_r, 0), (gt_z, KT)):
                        for j in range(KT):
                            nt = base + j
                            for kt in range(KT):
                                nc.tensor.matmul(
                                    dst[:, j, :],
                                    lhsT=wh_sb[:, kt, nt * 128:(nt + 1) * 128],
                                    rhs=hT_bf[:, kt, :],
                                    start=(kt == 0),
                                    stop=(kt == KT - 1),
                                )
                    tsl = slice(t * BL, (t + 1) * BL)
                    t1 = ew_pool.tile([128, KT, BL], F32, tag="t1")
                    nc.vector.tensor_add(t1, gt_n, bhn_sb)
                    pre_r = ew_pool.tile([128, KT, BL], F32, tag="pre_r")
                    nc.vector.tensor_add(pre_r, gt_r, xpt[:, 0:KT, tsl])
                    r = ew_pool.tile([128, KT, BL], F32, tag="r")
                    nc.scalar.activation(r, pre_r, AF.Sigmoid)
                    pre_z = ew_pool.tile([128, KT, BL], F32, tag="pre_z")
                    nc.vector.tensor_add(pre_z, gt_z, xpt[:, KT:2 * KT, tsl])
                    z = ew_pool.tile([128, KT, BL], F32, tag="z")
                    nc.scalar.activation(z, pre_z, AF.Sigmoid)
                    t2 = ew_pool.tile([128, KT, BL], F32, tag="t2")
                    nc.vector.tensor_mul(t2, r, t1)
                    t3 = ew_pool.tile([128, KT, BL], F32, tag="t3")
                    nc.vector.tensor_add(t3, t2, xpt[:, 2 * KT:3 * KT, tsl])
                    n_ = ew_pool.tile([128, KT, BL], F32, tag="n_")
                    nc.scalar.activation(n_, t3, AF.Tanh)
                    w = ew_pool.tile([128, KT, BL], F32, tag="w")
                    nc.vector.tensor_scalar(
                        w, z, -1.0, 1.0,
                        mybir.AluOpType.mult, mybir.AluOpType.add,
                    )
                    zh = ew_pool.tile([128, KT, BL], F32, tag="zh")
                    nc.vector.tensor_mul(zh, z, hprev_f)
                    t4 = ew_pool.tile([128, KT, BL], F32, tag="t4")
                    nc.vector.tensor_mul(t4, n_, w)
                    nc.vector.tensor_add(hT_bf, t4, zh)
                    nc.vector.tensor_add(hout[:, :, t, :], t4, zh)
                for kt in range(KT):
                    nc.sync.dma_start(
                        out=houtT[kt, :, bass.ds(bi * T, T), :],
                        in_=hout[:, kt, :, :],
                    )

            load_block(xpt_a, 0)
            nc.vector.tensor_copy(hout_b[:, :, T - 1, :], hz_f32)
            import concourse.mybir as _mb
            hint = (_mb.EngineType.PE, _mb.EngineType.DVE, _mb.EngineType.Activation)
            with tc.For_i(0, nblocks, 2, name="rec", hint_engines=hint,
                          staggered_reset=True) as bi:
                load_block(xpt_b, (bi + 1) * TB)
                compute_block(xpt_a, hout_a, hout_b, bi)
                load_block(xpt_a, (bi + 2) * TB)
                compute_block(xpt_b, hout_b, hout_a, bi + 1)

    nc.compile()
    return nc


def _host_prep(inputs):
    import ml_dtypes
    x = np.asarray(inputs["x"], dtype=np.float32)
    in_maps = []
    for c in range(NCORES):
        pfx = "f" if c < 4 else "b"
        q = c % 4
        bs = slice(q * BL, (q + 1) * BL)
        Wi = np.concatenate(
            [inputs[f"{pfx}_Wir"], inputs[f"{pfx}_Wiz"], inputs[f"{pfx}_Win"]],
            axis=1).astype(np.float32)
        Wh = np.concatenate(
            [inputs[f"{pfx}_Whr"], inputs[f"{pfx}_Whz"], inputs[f"{pfx}_Whn"]],
            axis=1).astype(np.float32)
        bias = np.concatenate([
            inputs[f"{pfx}_bir"] + inputs[f"{pfx}_bhr"],
            inputs[f"{pfx}_biz"] + inputs[f"{pfx}_bhz"],
            inputs[f"{pfx}_bin"],
        ]).astype(np.float32)
        bhn = np.asarray(inputs[f"{pfx}_bhn"], dtype=np.float32)

        xs = x[:, bs, :]
        xT = np.ascontiguousarray(xs.reshape(S * BL, KT, 128).transpose(1, 2, 0))
        in_maps.append({
            "xT": xT,
            "wi": np.ascontiguousarray(Wi.reshape(KT, 128, 3 * H)),
            "whT": np.ascontiguousarray(
                Wh.reshape(KT, 128, 3 * H)).astype(ml_dtypes.bfloat16),
            "bias_i": np.ascontiguousarray(bias.reshape(NT, 128).T),
            "bhn_bc": np.ascontiguousarray(
                np.broadcast_to(bhn.reshape(KT, 128).T[:, :, None],
                                (128, KT, BL))),
        })
    return in_maps


def kernel(**inputs):
    from concourse.bass_utils import run_bass_kernel_spmd

    if "nc" not in _CACHE:
        _CACHE["nc"] = _build_program()
    nc = _CACHE["nc"]

    in_maps = _host_prep(inputs)
    res = run_bass_kernel_spmd(nc, in_maps, core_ids=list(range(NCORES)))

    out = np.empty((S, B, 2 * H), dtype=np.float32)
    for c in range(NCORES):
        d = 0 if c < 4 else 1
        q = c % 4
        h = res.results[c]["houtT"]  # [KT, 128, S, BL]
        hf = h.transpose(2, 3, 0, 1).reshape(S, BL, H)
        out[:, q * BL:(q + 1) * BL, d * H:(d + 1) * H] = hf
    h_last = np.ascontiguousarray(out[-1:])
    return out, h_last


# revision 4
# speedup vs baseline: 1.0073x; 1.0073x over previous
"""BiGRU (nn_BiGRU) Trainium2 Bass kernel.

Sharding: 8 NeuronCores = 2 directions x 4 batch-quarters (8 sequences each).
The reference's "backward" cell also scans forward over time, so both
directions run the identical program on different weights/batch slices.

Per core:
  phase 1: XPT[n, t*8+b] = (x @ Wi + bi + bh(r,z))^T via fp32r GEMMs
           (hidden-major output so the recurrence elementwise runs with
           128 active partitions).
  phase 2: 2048-step GRU recurrence. Wh is the bf16 *stationary* matmul
           operand so each step's gate pre-activations come out already
           hidden-major in PSUM ([128, kt, batch]); no per-step transpose.
           The hidden state is carried twice: bf16 (next step's matmul
           operand, critical path) and fp32 (output + z*h term).
"""
import sys

sys.path.insert(0, "/opt/trn_rl_repo")
import numpy as np

S, B, I, H = 2048, 32, 512, 512
KT = 4    # contraction tiles of 128
NT = 12   # gate-dim tiles of 128 (3 gates x 4)
BL = 8    # sequences per core
T = 128   # recurrence steps per block
NCORES = 8

_CACHE = {}


def _build_program():
    import concourse.bass as bass
    import concourse.mybir as mybir
    import concourse.tile as tile
    from concourse import bacc
    from contextlib import ExitStack

    F32 = mybir.dt.float32
    F32R = mybir.dt.float32r
    BF16 = mybir.dt.bfloat16
    AF = mybir.ActivationFunctionType

    nblocks = S // T
    M = S * BL
    MCHUNK = 512
    nmchunks = M // MCHUNK
    TB = T * BL

    nc = bacc.Bacc("TRN2", target_bir_lowering=False, debug=False,
                   num_devices=NCORES)

    xT = nc.dram_tensor("xT", [KT, 128, M], F32R, kind="ExternalInput")
    wi = nc.dram_tensor("wi", [KT, 128, 3 * H], F32R, kind="ExternalInput")
    whT = nc.dram_tensor("whT", [KT, 128, 3 * H], BF16, kind="ExternalInput")
    bias_i = nc.dram_tensor("bias_i", [128, NT], F32, kind="ExternalInput")
    bhn_bc = nc.dram_tensor("bhn_bc", [128, KT, BL], F32, kind="ExternalInput")
    houtT = nc.dram_tensor("houtT", [KT, 128, S, BL], F32, kind="ExternalOutput")

    with tile.TileContext(nc) as tc, ExitStack() as stk:
        const_pool = stk.enter_context(tc.tile_pool(name="const", bufs=1))
        wh_sb = const_pool.tile([128, KT, 3 * H], BF16)
        bias_sb = const_pool.tile([128, NT], F32)
        bhn_sb = const_pool.tile([128, KT, BL], F32)
        hT_bf = const_pool.tile([128, KT, BL], BF16)
        hz_f32 = const_pool.tile([128, KT, BL], F32)
        for kt in range(KT):
            nc.sync.dma_start(out=wh_sb[:, kt, :], in_=whT[kt])
        nc.sync.dma_start(out=bias_sb, in_=bias_i[:])
        nc.sync.dma_start(out=bhn_sb, in_=bhn_bc[:])
        nc.vector.memset(hT_bf, 0.0)
        nc.vector.memset(hz_f32, 0.0)

        dram_pool = stk.enter_context(
            tc.tile_pool(name="dramp", bufs=1, space="DRAM"))
        xpt_dram = dram_pool.tile([NT, 128, M + 2 * TB], F32)

        # ---------------- phase 1: input projections ----------------
        with tc.tile_pool(name="p1wi", bufs=1) as wi_pool, \
             tc.tile_pool(name="p1x", bufs=3) as xin_pool, \
             tc.tile_pool(name="p1o", bufs=4) as xo_pool, \
             tc.tile_pool(name="p1ps", bufs=4, space="PSUM") as ps_pool:
            wi_sb = wi_pool.tile([128, KT, 3 * H], F32R)
            for kt in range(KT):
                nc.sync.dma_start(out=wi_sb[:, kt, :], in_=wi[kt])
            with tc.For_i(0, nmchunks, 4, name="p1") as ci:
                for u in range(4):
                    moff = (ci + u) * MCHUNK
                    xc = xin_pool.tile([128, KT, MCHUNK], F32R, tag="xc")
                    for kt in range(KT):
                        nc.sync.dma_start(
                            out=xc[:, kt, :], in_=xT[kt, :, bass.ds(moff, MCHUNK)]
                        )
                    for nt in range(NT):
                        ps = ps_pool.tile([128, MCHUNK], F32, tag="ps")
                        for kt in range(KT):
                            nc.tensor.matmul(
                                ps,
                                lhsT=wi_sb[:, kt, nt * 128:(nt + 1) * 128],
                                rhs=xc[:, kt, :],
                                start=(kt == 0),
                                stop=(kt == KT - 1),
                            )
                        xo = xo_pool.tile([128, MCHUNK], F32, tag="xo")
                        nc.vector.tensor_scalar_add(xo, ps, bias_sb[:, nt:nt + 1])
                        nc.sync.dma_start(
                            out=xpt_dram[nt, :, bass.ds(moff, MCHUNK)], in_=xo
                        )

        # ---------------- phase 2: recurrence ----------------
        with tc.tile_pool(name="p2x", bufs=1) as xpt_pool, \
             tc.tile_pool(name="p2o", bufs=1) as hout_pool, \
             tc.tile_pool(name="p2e", bufs=2) as ew_pool, \
             tc.tile_pool(name="p2ps", bufs=2, space="PSUM") as gps_pool:

            xpt_a = xpt_pool.tile([128, NT, TB], F32, tag="xpa")
            xpt_b = xpt_pool.tile([128, NT, TB], F32, tag="xpb")
            hout_a = hout_pool.tile([128, KT, T, BL], F32, tag="hoa")
            hout_b = hout_pool.tile([128, KT, T, BL], F32, tag="hob")

            def load_block(dst, start_col):
                for nt in range(NT):
                    nc.sync.dma_start(
                        out=dst[:, nt, :],
                        in_=xpt_dram[nt, :, bass.ds(start_col, TB)],
                    )

            def compute_block(xpt, hout, prev_hout, bi):
                for t in range(T):
                    hprev_f = (prev_hout[:, :, T - 1, :] if t == 0
                               else hout[:, :, t - 1, :])
                    gt_n = gps_pool.tile([128, KT, BL], F32, tag="gt_n")
                    gt_r = gps_pool.tile([128, KT, BL], F32, tag="gt_r")
                    gt_z = gps_pool.tile([128, KT, BL], F32, tag="gt_z")
                    for dst, base in ((gt_n, 2 * KT), (gt_r, 0), (gt_z, KT)):
                        for j in range(KT):
                            nt = base + j
                            for kt in range(KT):
                                nc.tensor.matmul(
                                    dst[:, j, :],
                                    lhsT=wh_sb[:, kt, nt * 128:(nt + 1) * 128],
                                    rhs=hT_bf[:, kt, :],
                                    start=(kt == 0),
                                    stop=(kt == KT - 1),
                                )
                    tsl = slice(t * BL, (t + 1) * BL)
                    t1 = ew_pool.tile([128, KT, BL], F32, tag="t1")
                    nc.vector.tensor_add(t1, gt_n, bhn_sb)
                    pre_r = ew_pool.tile([128, KT, BL], F32, tag="pre_r")
                    nc.vector.tensor_add(pre_r, gt_r, xpt[:, 0:KT, tsl])
                    r = ew_pool.tile([128, KT, BL], F32, tag="r")
                    nc.scalar.activation(r, pre_r, AF.Sigmoid)
                    pre_z = ew_pool.tile([128, KT, BL], F32, tag="pre_z")
                    nc.vector.tensor_add(pre_z, gt_z, xpt[:, KT:2 * KT, tsl])
                    z = ew_pool.tile([128, KT, BL], F32, tag="z")
                    nc.scalar.activation(z, pre_z, AF.Sigmoid)
                    t2 = ew_pool.tile([128, KT, BL], F32, tag="t2")
                    nc.vector.tensor_mul(t2, r, t1)
                    t3 = ew_pool.tile([128, KT, BL], F32, tag="t3")
                    nc.vector.tensor_add(t3, t2, xpt[:, 2 * KT:3 * KT, tsl])
                    n_ = ew_pool.tile([128, KT, BL], F32, tag="n_")
                    nc.scalar.activation(n_, t3, AF.Tanh)
                    w = ew_pool.tile([128, KT, BL], F32, tag="w")
                    nc.vector.tensor_scalar(
                        w, z, -1.0, 1.0,
                        mybir.AluOpType.mult, mybir.AluOpType.add,
                    )
                    zh = ew_pool.tile([128, KT, BL], F32, tag="zh")
                    nc.vector.tensor_mul(zh, z, hprev_f)
                    t4 = ew_pool.tile([128, KT, BL], F32, tag="t4")
                    nc.vector.tensor_mul(t4, n_, w)
                    nc.vector.tensor_add(hT_bf, t4, zh)
                    nc.vector.tensor_add(hout[:, :, t, :], t4, zh)
                for kt in range(KT):
                    nc.sync.dma_start(
                        out=houtT[kt, :, bass.ds(bi * T, T), :],
                        in_=hout[:, kt, :, :],
                    )

            load_block(xpt_a, 0)
            nc.vector.tensor_copy(hout_b[:, :, T - 1, :], hz_f32)
            import concourse.mybir as _mb
            hint = (_mb.EngineType.PE, _mb.EngineType.DVE, _mb.EngineType.Activation)
            with tc.For_i(0, nblocks, 2, name="rec", hint_engines=hint,
                          staggered_reset=True) as bi:
                load_block(xpt_b, (bi + 1) * TB)
                compute_block(xpt_a, hout_a, hout_b, bi)
                load_block(xpt_a, (bi + 2) * TB)
                compute_block(xpt_b, hout_b, hout_a, bi + 1)

    nc.compile()
    return nc


def _host_prep(inputs):
    import ml_dtypes
    x = np.asarray(inputs["x"], dtype=np.float32)
    in_maps = []
    for c in range(NCORES):
        pfx = "f" if c < 4 else "b"
        q = c % 4
        bs = slice(q * BL, (q + 1) * BL)
        Wi = np.concatenate(
            [inputs[f"{pfx}_Wir"], inputs[f"{pfx}_Wiz"], inputs[f"{pfx}_Win"]],
            axis=1).astype(np.float32)
        Wh = np.concatenate(
            [inputs[f"{pfx}_Whr"], inputs[f"{pfx}_Whz"], inputs[f"{pfx}_Whn"]],
            axis=1).astype(np.float32)
        bias = np.concatenate([
            inputs[f"{pfx}_bir"] + inputs[f"{pfx}_bhr"],
            inputs[f"{pfx}_biz"] + inputs[f"{pfx}_bhz"],
            inputs[f"{pfx}_bin"],
        ]).astype(np.float32)
        bhn = np.asarray(inputs[f"{pfx}_bhn"], dtype=np.float32)

        xs = x[:, bs, :]
        xT = np.ascontiguousarray(xs.reshape(S * BL, KT, 128).transpose(1, 2, 0))
        in_maps.append({
            "xT": xT,
            "wi": np.ascontiguousarray(Wi.reshape(KT, 128, 3 * H)),
            "whT": np.ascontiguousarray(
                Wh.reshape(KT, 128, 3 * H)).astype(ml_dtypes.bfloat16),
            "bias_i": np.ascontiguousarray(bias.reshape(NT, 128).T),
            "bhn_bc": np.ascontiguousarray(
                np.broadcast_to(bhn.reshape(KT, 128).T[:, :, None],
                                (128, KT, BL))),
        })
    return in_maps


def kernel(**inputs):
    from concourse.bass_utils import run_bass_kernel_spmd

    if "nc" not in _CACHE:
        _CACHE["nc"] = _build_program()
    nc = _CACHE["nc"]

    in_maps = _host_prep(inputs)
    res = run_bass_kernel_spmd(nc, in_maps, core_ids=list(range(NCORES)))

    out = np.empty((S, B, 2 * H), dtype=np.float32)
    for c in range(NCORES):
        d = 0 if c < 4 else 1
        q = c % 4
        h = res.results[c]["houtT"]  # [KT, 128, S, BL]
        hf = h.transpose(2, 3, 0, 1).reshape(S, BL, H)
        out[:, q * BL:(q + 1) * BL, d * H:(d + 1) * H] = hf
    h_last = np.ascontiguousarray(out[-1:])
    return out, h_last
